# revision 52
# baseline (speedup 1.0000x reference)
"""Trainium2 Bass kernel for a fake-quantized MLP (qlinear -> gelu -> qlinear).

Reference semantics (B,S,C,H = 32,1024,1024,4096):
    x2d = x.reshape(-1, C)
    h   = round(x2d/sx) @ round(w1/sw1).T * (sx*sw1) + b1 ;  s = max(amax,eps)/127
    g   = gelu(h, exact erf)
    y   = round(g/sh) @ round(w2/sw2).T * (sh*sw2) + b2

v6 strategy (data-parallel over rows, 8 cores):
  * HOST-SIDE TRANSPOSES: run() passes xT (per-core x2d shard transposed,
    [C, rows]), w1T (= w1.T, [C, H]) and w2T (= w2.T, [H, C]) so every
    matmul operand is loaded already in its contraction-on-partitions
    layout and quantized in place (ACT f32 MAGIC-round + DVE -MAGIC fp16
    cast straight into the operand tile).  No device transposes at all:
    v3/v4's XBAR DMA transposes (which corrupt each other when two are in
    flight on both HWDGE queues), PE identity transposes, psum copies,
    DRAM staging roundtrips and RAW fences are all gone.  Only g still
    stages through DRAM, produced in [h, rows] layout = exactly what
    phase B consumes (same-queue FIFO ordering on sync).
  * Sharded weight amax scans: each core also receives w1s/w2s (its 1/8
    slice) and scans 2MB instead of 16MB; global scales via 4-byte max
    AllReduces triggered BEFORE the x scan floods the DMA fabric (an AR
    under the 8-core scan load measured 86us vs ~15us on an idle fabric).
  * fp16 int domain everywhere off the PE (|q|<=127 exact in fp16), junk
    matmul chains keep the PE HAM clock gate at K=8 across DMA-bound
    stretches, strict emission-order discipline for the in-order queues.
  * PSUM: psJ(1) + psT(1, preduces) + psH(6) = 8 banks in phase A;
    psJ(1) + psT(1) + psY(5) = 7 in phase B.
"""

import sys

import numpy as np

try:
    import concourse.bass as bass
except ImportError:  # pragma: no cover
    sys.path.insert(0, "/opt/trn_rl_repo")
    import concourse.bass as bass

import concourse.mybir as mybir
from contextlib import ExitStack
import concourse.tile as tile
from concourse import masks
from concourse.bass_utils import run_bass_kernel_spmd

from concourse.bass import _add_dep_helper as _add_dep

F32 = mybir.dt.float32
FP16 = mybir.dt.float16
AF = mybir.ActivationFunctionType
ALU = mybir.AluOpType

QP = 127.0
EPS = 1e-6
MAGIC = 12582912.0  # 1.5 * 2**23: f32 round-to-nearest-even integer trick

# full problem shapes
B, S, C, H = 32, 1024, 1024, 4096
N_CORES = 8

A_CHUNKS = [1024, 1024, 1024, 1024]   # phase A row chunks
# junk matmuls pace at ~0.2-0.43us each (chained)
JUNK_PRE = 60        # PE warmers before the shard-scan preduces
JUNK_FRONT = 250     # PE warmers covering the x scan
JUNK_2 = 140         # PE warmers covering the x AR + chunk-0/w1-head quant
JUNK_MID = 85        # PE warmers covering the AllReduce(h) bubble
# phase B row chunks: small first to shorten the post-AllReduce critical path
PHB_CHUNKS = [128, 128, 256] + [512] * 7


def _split_matmul_waits(nc):
    """Walrus allows only ONE sync-wait slot per lowered instruction; peel
    extra waits onto same-engine NoOps inserted just before."""
    n_split = 0
    for f in nc.m.functions:
        for bb in f.blocks:
            insts = bb.instructions
            out = []
            changed = False
            for inst in insts:
                si = getattr(inst, "sync_info", None)
                if si is not None and si.on_wait and len(si.on_wait) > 1:
                    waits = list(si.on_wait)
                    for k, w in enumerate(waits[:-1]):
                        nop = mybir.InstNoOp(
                            name=f"{inst.name}-wsplit{k}", ins=[], outs=[]
                        )
                        nop.engine = inst.engine
                        nop.sync_info = mybir.SyncInfo(
                            on_wait=[w], on_update=[]
                        )
                        out.append(nop)
                    inst.sync_info = mybir.SyncInfo(
                        on_wait=[waits[-1]], on_update=list(si.on_update or [])
                    )
                    n_split += 1
                    changed = True
                out.append(inst)
            if changed:
                bb.instructions = out
    return n_split


def _dedup_ldweights(nc):
    """Drop an Ldweights whose weights AP is identical to the previous one
    on the PE stream (--enable-ldw-opt=false re-streams every stationary);
    keep its semaphore effects on a NoOp."""
    n = 0
    for f in nc.m.functions:
        for bb in f.blocks:
            insts = bb.instructions
            out = []
            last_key = None
            changed = False
            for inst in insts:
                if isinstance(inst, mybir.InstLdweights):
                    key = str(inst.ins[0])
                    if key == last_key:
                        si = getattr(inst, "sync_info", None)
                        if si is not None and (si.on_wait or si.on_update):
                            nop = mybir.InstNoOp(
                                name=inst.name + "-lw", ins=[], outs=[]
                            )
                            nop.engine = inst.engine
                            nop.sync_info = si
                            out.append(nop)
                        n += 1
                        changed = True
                        continue
                    last_key = key
                elif isinstance(inst, mybir.InstMatmult):
                    if inst.is_transpose or getattr(inst, "ldweights", None):
                        last_key = None
                out.append(inst)
            if changed:
                bb.instructions = out
    return n


def build_nc(rows=4096, c=C, h=H, n_cores=N_CORES, gelu="Gelu",
             split_waits=True, debug_taps=False):
    """Build the per-core SPMD Bass program."""
    assert sum(A_CHUNKS) == rows and c % 512 == 0 and h % 512 == 0
    nc = bass.Bass()

    ws_rows = h // n_cores           # w1 shard rows per core (512)
    w2s_rows = c // n_cores          # w2 shard rows per core (128)

    xT_in = nc.dram_tensor("xT", [c, rows], F32, kind="ExternalInput")
    w1T_in = nc.dram_tensor("w1T", [c, h], F32, kind="ExternalInput")
    w1s_in = nc.dram_tensor("w1s", [ws_rows, c], F32, kind="ExternalInput")
    b1_in = nc.dram_tensor("b1", [h], F32, kind="ExternalInput")
    w2T_in = nc.dram_tensor("w2T", [h, c], F32, kind="ExternalInput")
    w2s_in = nc.dram_tensor("w2s", [w2s_rows, h], F32, kind="ExternalInput")
    b2_in = nc.dram_tensor("b2", [c], F32, kind="ExternalInput")
    y_out = nc.dram_tensor("y", [rows, c], F32, kind="ExternalOutput")
    if debug_taps:
        g_out = nc.dram_tensor("g", [h, rows], FP16, kind="ExternalOutput")
        w1qT_out = nc.dram_tensor("w1qTo", [128, 8 * h], FP16,
                                  kind="ExternalOutput")
        w2qT_out = nc.dram_tensor("w2qTo", [128, 32 * c], FP16,
                                  kind="ExternalOutput")
        sc_out = nc.dram_tensor("scales", [128, 8], F32,
                                kind="ExternalOutput")

    ct = c // 128    # 8
    ht = h // 128    # 32
    groups = [list(range(n_cores))]
    a_chunks = []
    m0 = 0
    for ln in A_CHUNKS:
        a_chunks.append((m0, ln))
        m0 += ln
    phb_chunks = []
    m0 = 0
    for ln in PHB_CHUNKS:
        phb_chunks.append((m0, ln))
        m0 += ln
    assert m0 == rows

    with tile.TileContext(nc) as tc, ExitStack() as top:
        consts = top.enter_context(tc.tile_pool(name="consts", bufs=1))
        scal = top.enter_context(tc.tile_pool(name="scal", bufs=1))
        dram = top.enter_context(tc.tile_pool(name="dram", bufs=1, space="DRAM"))
        rowp = top.enter_context(tc.tile_pool(name="rowp", bufs=2))
        psJ = top.enter_context(tc.tile_pool(name="psJ", bufs=1, space="PSUM"))
        psT = top.enter_context(tc.tile_pool(name="psT", bufs=1, space="PSUM"))
        # w2qT lives from phase A (production) into phase B (consumption):
        # allocated below the pools that come and go, released at the end
        w2qT_pool = top.enter_context(tc.tile_pool(name="w2qT", bufs=1))
        w2qT = [
            w2qT_pool.tile([128, c], FP16, tag=f"w2qT{jb}", name=f"w2qT{jb}")
            for jb in range(ht)
        ]

        ident_f = consts.tile([128, 128], F32)
        masks.make_identity(nc, ident_f[:])
        # b1 as (128, ht): b1_sb[p, jb] = b1[jb*128 + p]
        b1_sb = consts.tile([128, ht], F32)
        nc.sync.dma_start(
            out=b1_sb[:], in_=b1_in.ap().rearrange("(a b) -> b a", b=128)
        )
        magic_b = consts.tile([128, 1], F32)
        nc.vector.memset(magic_b[:], MAGIC)
        junk_sb = consts.tile([128, 512], FP16)
        nc.vector.memset(junk_sb[:], 1.0)

        # DRAM scratch
        g_dram = dram.tile([h, rows], FP16)
        arw_in = dram.tile([1, 1], F32, tag="arwi")   # CC stream warmup
        arw_out = dram.tile([1, 1], F32, tag="arwo")
        arh_in = dram.tile([1, 1], F32, tag="arhi")
        arh_out = dram.tile([1, 1], F32, tag="arho")
        ar4_in = dram.tile([1, 4], F32, tag="ar4i")   # [-, w1max, w2max, -]
        ar4_out = dram.tile([1, 4], F32, tag="ar4o")
        arb_in = dram.tile([1, 1], F32, tag="arbi")   # xmax
        arb_out = dram.tile([1, 1], F32, tag="arbo")

        def _preduce_pe(acc, tag):
            """(128,1) partition max -> (1,1): one PE identity matmul
            (acc.T) + a DVE free-axis reduce.  Returns (scalar, mm_inst)."""
            pt = psT.tile([1, 128], F32, tag="pt", name=f"pt_{tag}")
            mmi = nc.tensor.matmul(pt[:], lhsT=acc[:], rhs=ident_f[:],
                                   start=True, stop=True)
            s = scal.tile([1, 1], F32, name=f"s1_{tag}")
            dve = nc.vector.tensor_reduce(
                out=s[:], in_=pt[:], axis=mybir.AxisListType.X, op=ALU.max
            )
            return s, mmi, dve

        def _fin_derive(b, name):
            s = scal.tile([128, 1], F32, name="s_" + name)
            nc.vector.tensor_scalar(
                out=s[:], in0=b[:], scalar1=EPS, scalar2=float(1.0 / QP),
                op0=ALU.max, op1=ALU.mult,
            )
            inv = scal.tile([128, 1], F32, name="inv_" + name)
            nc.vector.reciprocal(out=inv[:], in_=s[:])
            return s, inv

        def _derive_ar(bcast_src_dram, name, eng):
            """Post-AllReduce scale: broadcast-read the 4-byte result, then
            derive on DVE."""
            b = scal.tile([128, 1], F32, name=name + "_b")
            di = eng.dma_start(
                out=b[:], in_=bcast_src_dram.to_broadcast((128, 1))
            )
            s, inv = _fin_derive(b, name)
            return s, inv, di

        def _junk(n, tag, after=None):
            """PE warmers: n chained 512-wide accumulating matmuls on a
            constant tile; keeps the HAM clock gate at K=8 while real work
            is DMA/collective-bound.  One shared psum bank, never read.
            `after` pins the chain into its intended window: the Tile
            scheduler orders by dependency, not emission, and was observed
            hoisting unpinned junk chains out of the bubble they covered."""
            ps = psJ.tile([128, 512], F32, tag="junk", name=f"ps_{tag}")
            prev = after
            for i in range(n):
                mmi = nc.tensor.matmul(
                    ps[:], lhsT=junk_sb[:, 0:128], rhs=junk_sb[:],
                    start=(i == 0), stop=(i == n - 1),
                    skip_group_check=True,
                )
                if prev is not None:
                    _add_dep(mmi.ins, prev.ins, sync=False, reason="junk-order")
                prev = mmi
            return mmi

        def _fence(dep_insts, tag):
            """Tiny gpsimd op waiting on dep_insts (cross-queue RAW fence)."""
            f = rowp.tile([1, 1], F32, tag="fence", name=f"fence_{tag}")
            mi = nc.gpsimd.memset(f[:], 0.0)
            for dd in dep_insts:
                _add_dep(mi.ins, dd.ins, sync=True, reason="dram-raw-fence")
            return mi

        def _amax_tile(tl, acc, tag, dve_after=None):
            """DVE bulk |max| reduce + accumulation into acc."""
            r = rowp.tile([128, 1], F32, tag="red", name=f"r_{tag}")
            rd = nc.vector.tensor_reduce(
                out=r[:], in_=tl[:], axis=mybir.AxisListType.X, op=ALU.max,
                apply_absolute_value=True,
            )
            if dve_after is not None:
                _add_dep(rd.ins, dve_after.ins, sync=False,
                         reason="dve-order")
            return nc.vector.tensor_tensor(out=acc[:], in0=acc[:], in1=r[:],
                                           op=ALU.max)

        # long-lived operand tiles (until end of phase A)
        a_stack = ExitStack()
        w1qT_p = a_stack.enter_context(
            tc.tile_pool(name="w1qT", bufs=1, side="right")
        )
        w1qT = [
            w1qT_p.tile([128, h], FP16, tag=f"w1qT{i}", name=f"w1qT{i}")
            for i in range(ct)
        ]
        xqTs_pool = a_stack.enter_context(tc.tile_pool(name="xqTs", bufs=2))
        ldp = a_stack.enter_context(tc.tile_pool(name="ldp", bufs=3))

        def new_xqTs(mc):
            cm0, clen = a_chunks[mc]
            return [
                xqTs_pool.tile([128, clen], FP16, tag=f"xqT{cb}",
                               name=f"xqT{mc}_{cb}")
                for cb in range(ct)
            ]

        def quant_tile(src_slice, dst_slice, inv_s, eng, tag, dep=None,
                       dve_after=None):
            """Load a [128, 1024] f32 block, MAGIC-round on ACT with scale
            inv_s, cast to fp16 ints on DVE straight into the operand."""
            t = ldp.tile([128, 1024], F32, tag="ld", name=f"ld_{tag}")
            ld = eng.dma_start(out=t[:], in_=src_slice)
            if dep is not None:
                _add_dep(ld.ins, dep.ins, sync=False, reason="q-order")
            nc.scalar.activation(
                out=t[:], in_=t[:], func=AF.Identity,
                bias=magic_b[:], scale=inv_s[:],
            )
            cv = nc.vector.tensor_scalar_add(out=dst_slice, in0=t[:],
                                             scalar1=-MAGIC)
            if dve_after is not None:
                _add_dep(cv.ins, dve_after.ins, sync=False,
                         reason="dve-order")
            return ld

        # ---------------- front ----------------
        xmax = scal.tile([128, 1], F32)
        nc.vector.memset(xmax[:], 0.0)
        wmax1 = scal.tile([128, 1], F32)
        nc.vector.memset(wmax1[:], 0.0)
        wmax2 = scal.tile([128, 1], F32)
        nc.vector.memset(wmax2[:], 0.0)

        # small junk so the HAM window warms while shard scans run
        _junk(JUNK_PRE, "junkP")

        # warm the collectives stream: the first CC op pays a large cold
        # cost (~60us observed) -- eat it at t=0 on a dummy
        cc_warm = nc.gpsimd.collective_compute(
            "AllReduce", ALU.max, replica_groups=groups,
            ins=[arw_in.opt()], outs=[arw_out.opt()],
        )

        # sharded weight amax scans FIRST (tiny; at the queue heads)
        for t in range(ws_rows // 128):          # 4 tiles of w1 shard
            eng = nc.sync if t % 2 == 0 else nc.scalar
            wt = ldp.tile([128, 1024], F32, tag="ld", name=f"w1sh{t}")
            eng.dma_start(out=wt[:], in_=w1s_in[t * 128:(t + 1) * 128, :])
            _amax_tile(wt, wmax1, f"w1s{t}")
        for t in range(2):                       # 2 row-slices of w2 shard
            eng = nc.sync if t % 2 == 0 else nc.scalar
            wt = ldp.tile([128, 2048], F32, tag="ld2", name=f"w2sh{t}",
                          bufs=3)
            eng.dma_start(out=wt[:], in_=w2s_in[:, t * 2048:(t + 1) * 2048])
            _amax_tile(wt, wmax2, f"w2s{t}")
        w1m_s, mm_w1m, _ = _preduce_pe(wmax1, "w1m")
        w2m_s, mm_w2m, dve_w2m = _preduce_pe(wmax2, "w2m")

        # weight-scale AllReduce (combined w1+w2, 8 bytes): this is also
        # the CC-stream warmup op (pays the cold-start; done by ~50us,
        # needed at ~90)
        s4 = scal.tile([1, 4], F32)
        nc.vector.memset(s4[:], 0.0)
        nc.vector.tensor_copy(out=s4[0:1, 1:2], in_=w1m_s[:])
        nc.vector.tensor_copy(out=s4[0:1, 2:3], in_=w2m_s[:])
        nc.gpsimd.dma_start(out=ar4_in[:], in_=s4[:])
        cc4 = nc.gpsimd.collective_compute(
            "AllReduce", ALU.max, replica_groups=groups,
            ins=[ar4_in.opt()], outs=[ar4_out.opt()],
        )
        _add_dep(cc4.ins, cc_warm.ins, sync=False, reason="gps-order")
        b4 = scal.tile([128, 4], F32)
        d4 = nc.gpsimd.dma_start(out=b4[:], in_=ar4_out.to_broadcast((128, 4)))
        _add_dep(d4.ins, cc4.ins, sync=False, reason="gps-order")

        # x amax scan owns both HWDGE queues; full-row [128, 2048] tiles
        # (8KB contiguous per-partition segments -- the 4KB-segment version
        # scanned at only ~100GB/s).  The w1-head loads interleave into the
        # scan tail: their ACTs/casts only need sw1 (ready early), so they
        # are fully processed before the x AllReduce fires.
        _junk(JUNK_FRONT, "junkF", after=mm_w1m)
        for i in range(16):
            cb, rg = i % ct, i // ct
            eng = nc.sync if i % 2 == 0 else nc.scalar
            xt = ldp.tile([128, 2048], F32, tag="ld2", name=f"xs{i}", bufs=3)
            eng.dma_start(
                out=xt[:],
                in_=xT_in[cb * 128:(cb + 1) * 128,
                          rg * 2048:(rg + 1) * 2048],
            )
            _amax_tile(xt, xmax, f"x{i}",
                       dve_after=dve_w2m if i == 0 else None)
        xm_s, mm_xm, dve_xm = _preduce_pe(xmax, "xm")

        # x-scale AllReduce fired at scan end on an idle fabric: the
        # chunk-0 loads are hard-gated on its completion
        sb = scal.tile([1, 1], F32)
        nc.vector.tensor_copy(out=sb[:], in_=xm_s[:])
        nc.sync.dma_start(out=arb_in[:], in_=sb[:])
        ccb = nc.gpsimd.collective_compute(
            "AllReduce", ALU.max, replica_groups=groups,
            ins=[arb_in.opt()], outs=[arb_out.opt()],
        )
        _add_dep(ccb.ins, d4.ins, sync=False, reason="gps-order")
        # derive sw1/sw2 here: these DVE ops wait on the weight-AR read
        # (~100us) and would block the x amaxes if emitted any earlier
        sw1, inv_sw1 = _fin_derive(b4[:, 1:2], "w1")
        sw2, inv_sw2 = _fin_derive(b4[:, 2:3], "w2")

        # w1 head + w1 quarter-1 AFTER the x-AR trigger: their DVE casts
        # are explicitly ordered after the xm reduce (the scheduler was
        # observed interleaving them ahead of it, delaying the x AR)
        for wcb in range(ct):
            quant_tile(
                w1T_in[wcb * 128:(wcb + 1) * 128, 0:1024],
                w1qT[wcb][:, 0:1024], inv_sw1,
                nc.sync if wcb % 2 == 0 else nc.scalar, f"w1h{wcb}",
                dve_after=dve_xm,
            )
        for wcb in range(ct):
            quant_tile(
                w1T_in[wcb * 128:(wcb + 1) * 128, 1024:2048],
                w1qT[wcb][:, 1024:2048], inv_sw1,
                nc.sync if wcb % 2 == 0 else nc.scalar, f"w1q1_{wcb}",
                dve_after=dve_xm,
            )
        sx, inv_sx, d_x = _derive_ar(arb_out, "x", nc.gpsimd)
        _add_dep(d_x.ins, ccb.ins, sync=False, reason="gps-order")
        sxw1 = scal.tile([128, 1], F32)
        nc.vector.tensor_tensor(out=sxw1[:], in0=sx[:], in1=sw1[:],
                                op=ALU.mult)

        xqTs0 = new_xqTs(0)
        for cb in range(ct):
            ld = quant_tile(
                xT_in[cb * 128:(cb + 1) * 128, 0:1024],
                xqTs0[cb][:], inv_sx,
                nc.sync if cb % 2 == 0 else nc.scalar, f"x0_{cb}",
            )
            if cb < 2:
                _add_dep(ld.ins, ccb.ins, sync=True, reason="ar-quiet")

        # ---------------- phase A ----------------
        hmax = scal.tile([128, 1], F32)
        nc.vector.memset(hmax[:], 0.0)
        g3_stores = []

        with ExitStack() as pha:
            psH = pha.enter_context(
                tc.tile_pool(name="psH", bufs=6, space="PSUM")
            )
            gS = pha.enter_context(tc.tile_pool(name="gS", bufs=2))

            # PE warmers between the front preduces and the first matmul
            _junk(JUNK_2, "junk2", after=mm_xm)

            all_xqTs = [xqTs0] + [new_xqTs(mc) for mc in range(1, 4)]

            # interleave schedules -------------------------------------
            # w1 tail: quarters 2-3 x 8 cb = 16 tiles, 2 per jb over
            # jb 1..8 of chunk 0 (quarters 0-1 were emitted in the front;
            # consumption: jb needs quarter jb//8)
            w1_sched = {}
            k = 8
            for jb in range(1, 9):
                for _ in range(2):
                    q, cb = 1 + k // 8, k % 8
                    w1_sched.setdefault((0, jb), []).append((q, cb))
                    k += 1
            # w2: 32 jb-blocks, even jbs of chunks 1 and 2
            w2_sched = {}
            for i in range(16):
                w2_sched[(1, 2 * i)] = i
                w2_sched[(2, 2 * i)] = 16 + i
            # next-chunk x quant: 8 tiles at jb = 14,16,..,28 of prev chunk
            xq_sched = {}
            for mc in range(3):
                for i in range(8):
                    xq_sched[(mc, 14 + 2 * i)] = i

            for mc, (cm0, clen) in enumerate(a_chunks):
                n_ms = clen // 512
                xqTs = all_xqTs[mc]
                for jb in range(ht):
                    phs = [
                        psH.tile([128, 512], F32, tag="psH",
                                 name=f"psH{mc}_{jb}_{i}")
                        for i in range(n_ms)
                    ]
                    prev = None
                    for cb in range(ct):
                        for ms in range(n_ms):
                            mmi = nc.tensor.matmul(
                                phs[ms][:],
                                lhsT=w1qT[cb][:, jb * 128:(jb + 1) * 128],
                                rhs=xqTs[cb][:, ms * 512:(ms + 1) * 512],
                                start=(cb == 0),
                                stop=(cb == ct - 1),
                            )
                            if prev is not None:
                                _add_dep(mmi.ins, prev.ins, sync=False,
                                         reason="ldw-order")
                            prev = mmi
                    g = gS.tile([128, clen], FP16, tag="gS",
                                name=f"g{mc}_{jb}")
                    for ms in range(n_ms):
                        nc.scalar.activation(
                            out=g[:, ms * 512:(ms + 1) * 512],
                            in_=phs[ms][:], func=getattr(AF, gelu),
                            bias=b1_sb[:, jb:jb + 1], scale=sxw1[:],
                        )
                    _amax_tile(g, hmax, f"g{mc}_{jb}")
                    # chunk-3 g stores go on scalar: the sync queue is then
                    # EMPTY at phase-A end, so the phase-B hl prefetches and
                    # sh broadcast are not stuck behind the g tail.  Phase-B
                    # hl loads that read chunk-3 rows get explicit fences.
                    g_eng = nc.scalar if mc == 3 else nc.sync
                    st_g = g_eng.dma_start(
                        out=g_dram[jb * 128:(jb + 1) * 128, cm0:cm0 + clen],
                        in_=g[:],
                    )
                    if mc == 3:
                        g3_stores.append(st_g)
                    # interleaved producers:
                    for (q, cb) in w1_sched.get((mc, jb), ()):
                        quant_tile(
                            w1T_in[cb * 128:(cb + 1) * 128,
                                   q * 1024:(q + 1) * 1024],
                            w1qT[cb][:, q * 1024:(q + 1) * 1024],
                            inv_sw1,
                            nc.sync if cb % 2 == 0 else nc.scalar,
                            f"w1_{q}_{cb}",
                        )
                    if (mc, jb) in w2_sched:
                        t = w2_sched[(mc, jb)]
                        quant_tile(
                            w2T_in[t * 128:(t + 1) * 128, :],
                            w2qT[t][:], inv_sw2,
                            nc.scalar, f"w2_{t}",
                        )
                    if (mc, jb) in xq_sched:
                        cb = xq_sched[(mc, jb)]
                        nmc = mc + 1
                        nm0 = a_chunks[nmc][0]
                        quant_tile(
                            xT_in[cb * 128:(cb + 1) * 128,
                                  nm0:nm0 + 1024],
                            all_xqTs[nmc][cb][:], inv_sx,
                            nc.sync if cb % 2 == 0 else nc.scalar,
                            f"x{nmc}_{cb}",
                        )

        if debug_taps:
            for i in range(ct):
                nc.gpsimd.dma_start(
                    out=w1qT_out[:, i * h:(i + 1) * h], in_=w1qT[i][:]
                )

        a_stack.close()

        # ---------------- phase B ----------------
        with ExitStack() as phb:
            psY = phb.enter_context(
                tc.tile_pool(name="psY", bufs=5, space="PSUM")
            )

            # h scale AllReduce: PE preduce (runs right after the last
            # phase-A matmul), scalar-queue write (idle by then), gpsimd
            # trigger (its FIFO is clear)
            hm_s, mm_hm, _ = _preduce_pe(hmax, "hm")
            # sync queue is empty at phase-A end (chunk-3 g went to scalar)
            nc.sync.dma_start(out=arh_in[:], in_=hm_s[:])
            nc.gpsimd.collective_compute(
                "AllReduce", ALU.max, replica_groups=groups,
                ins=[arh_in.opt()], outs=[arh_out.opt()],
            )
            hld = phb.enter_context(tc.tile_pool(name="hld", bufs=2))
            hq1p = phb.enter_context(tc.tile_pool(name="hq1p", bufs=3))
            yS = phb.enter_context(tc.tile_pool(name="yS", bufs=3))
            b2p = phb.enter_context(tc.tile_pool(name="b2p", bufs=1))

            # PE warmers across the AllReduce bubble
            _junk(JUNK_MID, "junkM", after=mm_hm)

            b2_b = b2p.tile([128, c], F32)
            nc.sync.dma_start(
                out=b2_b[:],
                in_=b2_in.ap().rearrange("(o a) -> o a", o=1).to_broadcast(
                    (128, c)),
            )
            # prefetch the first two h chunks before the sh broadcast so
            # the in-order sync queue does not hold them behind it
            hl_tiles = {}
            for ci in (0, 1):
                m0, mlen = phb_chunks[ci]
                hl = hld.tile([128, ht, 512], FP16, tag="hld",
                              name=f"hl{ci}")
                nc.sync.dma_start(
                    out=hl[:, :, 0:mlen],
                    in_=g_dram[:, m0:m0 + mlen].rearrange(
                        "(a p) m -> p a m", p=128),
                )
                hl_tiles[ci] = hl

            sh, inv_sh, _ = _derive_ar(arh_out, "h", nc.sync)
            shw2 = scal.tile([128, 1], F32)
            nc.vector.tensor_tensor(out=shw2[:], in0=sh[:], in1=sw2[:],
                                    op=ALU.mult)

            # chunk-3 g stores are on the scalar queue; hl loads (sync) of
            # rows >= 3072 need an explicit cross-queue RAW fence
            g3fence = _fence(g3_stores, "g3")

            for ci, (m0, mlen) in enumerate(phb_chunks):
                if ci in hl_tiles:
                    hl = hl_tiles.pop(ci)
                else:
                    hl = hld.tile([128, ht, 512], FP16, tag="hld",
                                  name=f"hl{ci}")
                    ldh = nc.sync.dma_start(
                        out=hl[:, :, 0:mlen],
                        in_=g_dram[:, m0:m0 + mlen].rearrange(
                            "(a p) m -> p a m", p=128),
                    )
                    if m0 + mlen > a_chunks[3][0]:
                        _add_dep(ldh.ins, g3fence.ins, sync=True,
                                 reason="g3-raw")
                for j4 in range(ht // 4):
                    sl = hl[:, j4 * 4:(j4 + 1) * 4, 0:mlen]
                    hq1 = hq1p.tile([128, 4, 512], F32, tag="hq1",
                                    name=f"hq1_{ci}_{j4}")
                    nc.scalar.activation(
                        out=hq1[:, :, 0:mlen], in_=sl, func=AF.Identity,
                        bias=magic_b[:], scale=inv_sh[:],
                    )
                    nc.vector.tensor_scalar_add(
                        out=sl, in0=hq1[:, :, 0:mlen], scalar1=-MAGIC
                    )
                for ms in range(mlen // 128):
                    psa = psY.tile([128, 512], F32, tag="psY",
                                   name=f"psa{ci}_{ms}")
                    psb = psY.tile([128, 512], F32, tag="psY",
                                   name=f"psb{ci}_{ms}")
                    prev = None
                    for jb in range(ht):
                        lt = hl[:, jb:jb + 1, ms * 128:(ms + 1) * 128]
                        for ob, pso in ((0, psa), (1, psb)):
                            mmi = nc.tensor.matmul(
                                pso[:], lhsT=lt,
                                rhs=w2qT[jb][:, ob * 512:(ob + 1) * 512],
                                start=(jb == 0), stop=(jb == ht - 1),
                            )
                            if prev is not None:
                                _add_dep(mmi.ins, prev.ins, sync=False,
                                         reason="ldw-order")
                            prev = mmi
                    yt = yS.tile([128, c], F32, tag="yS", name=f"y{ci}_{ms}")
                    nc.vector.scalar_tensor_tensor(
                        out=yt[:, 0:512], in0=psa[:], scalar=shw2[:],
                        in1=b2_b[:, 0:512], op0=ALU.mult, op1=ALU.add,
                    )
                    nc.vector.scalar_tensor_tensor(
                        out=yt[:, 512:1024], in0=psb[:], scalar=shw2[:],
                        in1=b2_b[:, 512:1024], op0=ALU.mult, op1=ALU.add,
                    )
                    r0 = m0 + ms * 128
                    nc.scalar.dma_start(out=y_out[r0:r0 + 128, :], in_=yt[:])

            if debug_taps:
                for jb in range(ht):
                    nc.gpsimd.dma_start(
                        out=w2qT_out[:, jb * c:(jb + 1) * c], in_=w2qT[jb][:]
                    )
                nc.gpsimd.dma_start(out=sc_out[:, 0:1], in_=sx[:])
                nc.gpsimd.dma_start(out=sc_out[:, 1:2], in_=sw1[:])
                nc.gpsimd.dma_start(out=sc_out[:, 2:3], in_=sw2[:])
                nc.gpsimd.dma_start(out=sc_out[:, 3:4], in_=sh[:])
                nc.gpsimd.dma_start(out=g_out.ap(), in_=g_dram[:])

    if split_waits:
        _split_matmul_waits(nc)
        _dedup_ldweights(nc)
    return nc


_CACHED = {}


def _get_nc(rows, c, h, n_cores, gelu, debug_taps=False):
    key = (rows, c, h, n_cores, gelu, debug_taps)
    if key not in _CACHED:
        _CACHED[key] = build_nc(rows=rows, c=c, h=h, n_cores=n_cores,
                                gelu=gelu, debug_taps=debug_taps)
    return _CACHED[key]


def run(inputs, trace=False, gelu="Gelu", n_cores=N_CORES, debug_taps=False):
    x = np.asarray(inputs["x"], np.float32)
    w1 = np.ascontiguousarray(np.asarray(inputs["w1"], np.float32))
    b1 = np.ascontiguousarray(np.asarray(inputs["b1"], np.float32))
    w2 = np.ascontiguousarray(np.asarray(inputs["w2"], np.float32))
    b2 = np.ascontiguousarray(np.asarray(inputs["b2"], np.float32))
    b_, s_, c_ = x.shape
    h_ = w1.shape[0]
    x2d = np.ascontiguousarray(x.reshape(-1, c_))
    rows = x2d.shape[0] // n_cores
    ws = h_ // n_cores
    w2s_n = c_ // n_cores
    nc = _get_nc(rows, c_, h_, n_cores, gelu, debug_taps)
    w1T = np.ascontiguousarray(w1.T)          # [C, H]
    w2T = np.ascontiguousarray(w2.T)          # [H, C]
    in_maps = [
        {
            "xT": np.ascontiguousarray(x2d[i * rows:(i + 1) * rows].T),
            "w1T": w1T,
            "w1s": np.ascontiguousarray(w1[i * ws:(i + 1) * ws]),
            "b1": b1,
            "w2T": w2T,
            "w2s": np.ascontiguousarray(w2[i * w2s_n:(i + 1) * w2s_n]),
            "b2": b2,
        }
        for i in range(n_cores)
    ]
    res = run_bass_kernel_spmd(nc, in_maps, list(range(n_cores)), trace=trace)
    y2d = np.concatenate([r["y"] for r in res.results], axis=0)
    return y2d.reshape(b_, s_, c_).astype(np.float32), res


def kernel(x, w1, b1, w2, b2):
    y, _ = run({"x": x, "w1": w1, "b1": b1, "w2": w2, "b2": b2})
    return y
